# revision 74
# baseline (speedup 1.0000x reference)
"""MoE (BruteForceMoELinear) Trainium2 kernel — bf16 expert-parallel.

Strategy: expert-parallel across 8 NeuronCores.  The host dispatches
token rows by `gate_idx` (stable sort), folds the per-row gate score
into the activations (scores >= 0 commute through ReLU), pads each
expert's batch to capacity C, and hands core e bf16-packed operands.

Per-core compute: y_e^T = W2_e @ relu(W1_e @ x_e^T), bf16 matmuls with
fp32 PSUM accumulation.  Tokens split into a big chunk A (<=512 cols)
and a small remainder B.  GEMM1 opens ko-major over the first FO1
f-groups so the PE consumes each (W1-ko, x-ko) row-DMA the moment it
lands; W1-ko and x-ko are packed into a single DRAM row per ko to
minimize per-DMA descriptor-generation serialization.  The rest runs
fo-major against streamed W1, with B's tiny groups woven between A
groups.

Output tail: a plain HWDGE store pays ~625ns descriptor-gen + ~650ns
DGE->DMA latency after the final eviction, so the last d-group is
drained through SWDGE instead — dma_scatter_add descriptors are
pre-generated early on the Pool engine (prepare_only, one queue per
piece, src-ordering edge stripped so desc-gen runs off the critical
path), and each piece fires with a cheap trigger_dma the moment its
PSUM eviction lands (runtime pre-zeroes outputs, so add == store).
The scatter idx tile replicates the 16-partition wrap across all
partition groups because queue k's Q7 core reads channel stripe
[32k, 32k+32).  The last d-group is split 352/128/32, the two late
pieces stored as bf16 (halves their sub-512B descriptor payloads) so
the final eviction and store are as short as the fixed semaphore
costs allow.  The B remainder and earlier d-groups finish long before the
end on the normal HWDGE path; one early x/W1 row goes through a
Pool-engine (SWDGE) copy to take a gen slot off the HWDGE ring.
"""

import numpy as np
import ml_dtypes

import os

NUM_EXPERT = 8
N_CORES = 8
P = 128
FO1 = int(os.environ.get("K_FO1", "5"))  # ko-major head fo-groups
_CUT = int(os.environ.get("K_CUT", "2"))     # W1 cols in first DMA piece
_S1 = int(os.environ.get("K_S1", "128"))     # final scatter piece cols

_CACHE = {}


def _chunks_for(C):
    if C <= 512:
        return [C]
    assert C <= 1024
    return [512, C - 512]


def _build(C, KO, FO, repeat=1):
    key = (C, KO, FO, repeat)
    if key in _CACHE:
        return _CACHE[key]

    import concourse.mybir as mybir
    import concourse.tile as tile
    from concourse import bacc
    f32 = mybir.dt.float32
    bf16 = mybir.dt.bfloat16
    i16 = mybir.dt.int16
    chunks = _chunks_for(C)
    TA = chunks[0]
    TB = chunks[1] if len(chunks) > 1 else 0
    nfo1 = min(FO1, FO)
    FOB = FO - nfo1
    RS = TA + nfo1 * P           # row stride: x-ko | w1a-ko
    XWN = KO * RS + KO * TB      # + xB appended at the end
    use_sw = (TA % 128 == 0) and TA >= 2 * _S1  # SWDGE tail path
    # last-d-group piece sizes (queues 0/1/2); earlier pieces evict and
    # store while later pieces still run on the PE, so the final piece
    # stays small and its transfer finds the DMA engines idle.
    _SS = os.environ.get("K_SS", "352,128,32")
    S0, S1A, S1B = (int(v) for v in _SS.split(","))
    if S0 + S1A + S1B != TA or min(S0, S1A) < 64 or S1B < 32:
        S0, S1A, S1B = TA - 128, 64, 64

    nc = bacc.Bacc("TRN2", target_bir_lowering=False, debug=False,
                   num_devices=N_CORES,
                   num_swdge_queues=3 if use_sw else 1)

    xw = nc.dram_tensor("xw", (P, XWN), bf16, kind="ExternalInput")
    w1b = nc.dram_tensor("w1b", (P, FOB, KO * P), bf16, kind="ExternalInput")
    w2 = nc.dram_tensor("w2", (P, KO, FO * P), bf16, kind="ExternalInput")
    yt = nc.dram_tensor("yt", (P, KO * C), bf16, kind="ExternalOutput")
    if use_sw:
        sidx = nc.dram_tensor("sidx", (P, 8), i16, kind="ExternalInput")
        ylast = nc.dram_tensor("ylast", (P, TA), f32, kind="ExternalOutput")
        ylastb = nc.dram_tensor("ylastb", (P, 256), bf16,
                                kind="ExternalOutput")

    with tile.TileContext(nc) as tc:
        with tc.tile_pool(name="wpool", bufs=1) as wpool, \
             tc.tile_pool(name="ypool", bufs=4) as ypool, \
             tc.tile_pool(name="psA", bufs=6, space="PSUM") as psA, \
             tc.tile_pool(name="psB", bufs=2, space="PSUM") as psB:

            xwsb = wpool.tile([P, XWN], bf16, name="xwsb")
            w1bsb = (wpool.tile([P, FOB, KO * P], bf16, name="w1bsb")
                     if FOB else None)
            w2sb = wpool.tile([P, KO, FO * P], bf16, name="w2sb")
            hA = wpool.tile([P, FO, TA], bf16, name="hA")
            hB = wpool.tile([P, FO, TB], bf16, name="hB") if TB else None
            if use_sw:
                sidxsb = wpool.tile([P, 8], i16, name="sidxsb")
                ysl0 = wpool.tile([P, 1, S0], f32, name="ysl0")
                ysl1a = wpool.tile([P, 1, S1A], bf16, name="ysl1a")
                ysl1b = wpool.tile([P, 1, S1B], bf16, name="ysl1b")

            def xA_ap(ko):
                return xwsb[:, ko * RS:ko * RS + TA]

            def xB_ap(ko):
                return xwsb[:, KO * RS + ko * TB:KO * RS + (ko + 1) * TB]

            def w1_ap(f, ko):
                if f < nfo1:
                    off = ko * RS + TA + f * P
                    return xwsb[:, off:off + P]
                return w1bsb[:, f - nfo1, ko * P:(ko + 1) * P]

            # --- DMAs: emission order == consumption order -------------
            cut = TA + _CUT * P if nfo1 >= _CUT else RS
            _POOLB = int(os.environ.get("K_POOLB", "1"))
            nc.sync.dma_start(xwsb[:, 0:cut], xw.ap()[:, 0:cut])
            if cut < RS:
                # Pool-engine (SWDGE) copy: its descriptor-gen runs in
                # parallel with the shared HWDGE ring, freeing a gen slot
                # so the ko1 row lands before phase 1 consumes it.
                eng = nc.gpsimd if _POOLB else nc.sync
                eng.dma_start(xwsb[:, cut:RS], xw.ap()[:, cut:RS])
            KOSPLIT = int(os.environ.get("K_KOSPLIT", "0"))
            for ko in range(1, KO):
                hi = (ko + 1) * RS if ko < KO - 1 else XWN
                lo = ko * RS
                if ko <= KOSPLIT and cut < RS:
                    nc.sync.dma_start(xwsb[:, lo:lo + cut],
                                      xw.ap()[:, lo:lo + cut])
                    lo += cut
                nc.sync.dma_start(xwsb[:, lo:hi], xw.ap()[:, lo:hi])
            fo = 0
            while fo < FOB:
                hi = min(fo + int(os.environ.get("K_W1B", "4")), FOB)
                nc.sync.dma_start(w1bsb[:, fo:hi, :], w1b.ap()[:, fo:hi, :])
                fo = hi
            if use_sw:
                nc.sync.dma_start(sidxsb[:], sidx.ap()[:])
            nc.sync.dma_start(w2sb[:, 0:2, :], w2.ap()[:, 0:2, :])
            nc.sync.dma_start(w2sb[:, 2:KO, :], w2.ap()[:, 2:KO, :])
            if use_sw:
                # No explicit zeroing of ylast: both execution paths
                # (native run_neff and the PJRT redirect) pre-zero
                # ExternalOutput buffers, so scatter-add == store.
                sem0 = nc.alloc_semaphore("sdma0")
                sem1 = nc.alloc_semaphore("sdma1")
                sem2 = nc.alloc_semaphore("sdma2")
                prep0 = nc.gpsimd.dma_scatter_add(
                    ylast.ap()[:, 0:S0], ysl0[:], sidxsb[:], P, P, S0,
                    elem_step=TA, prepare_only=True, sem=sem0, queue_num=0,
                    single_packet=False)
                prep1 = nc.gpsimd.dma_scatter_add(
                    ylastb.ap()[:, 0:S1A], ysl1a[:], sidxsb[:], P, P,
                    S1A, elem_step=256, prepare_only=True, sem=sem1,
                    queue_num=1, single_packet=False)
                prep2 = nc.gpsimd.dma_scatter_add(
                    ylastb.ap()[:, 128:128 + S1B], ysl1b[:], sidxsb[:],
                    P, P, S1B, elem_step=256, prepare_only=True, sem=sem2,
                    queue_num=2, single_packet=False)

            def evict1(dst, src, use_act):
                if use_act:
                    nc.scalar.activation(dst, src,
                                         mybir.ActivationFunctionType.Relu)
                else:
                    nc.vector.tensor_scalar_max(dst, src, 0.0)

            # Keep-warm bridge: the cost model resets the PE p-state
            # anchor when the PE idles more than ~0.8us, and the ramp to
            # full rate takes 3us from the anchor.  Emit a chain of tiny
            # matmuls, each gated by a ~0.6us Pool-engine memset, so PE
            # activity recurs every <0.7us until the first real matmul
            # (~3.5us, after the row-0 DMA) — which then runs full-rate.
            warm = wpool.tile([P, 16], bf16, name="warm")
            NPACE = 18
            pace = wpool.tile([P, (NPACE + 1) * 16], bf16, name="pace")
            bconst = nc.const_aps.aps[(mybir.dt.bfloat16, 1.0)]
            nc.tensor.ldweights(bconst)
            nc.vector.memset(warm[:], 0.0)
            nc.vector.memset(pace[:, 0:16], 0.0)
            wps = psB.tile([P, 16], f32, name="wps", tag="pB")
            nc.tensor.matmul(wps[0:16, :], warm[:], warm[:],
                             start=True, stop=True)
            for k in range(NPACE):
                nc.vector.tensor_scalar_add(
                    pace[:, (k + 1) * 16:(k + 2) * 16],
                    pace[:, k * 16:(k + 1) * 16], 0.0)
                nc.tensor.matmul(
                    wps[0:16, :], warm[:],
                    pace[:, (k + 1) * 16:(k + 2) * 16],
                    start=True, stop=True)

            # --- GEMM1 phase 1: ko-major over fo 0..nfo1 on chunk A ----
            p1s = [psA.tile([P, TA], f32, name=f"p1f{f}", tag="pA")
                   for f in range(nfo1)]
            for ko in range(KO):
                for f in range(nfo1):
                    nc.tensor.matmul(p1s[f][:], w1_ap(f, ko), xA_ap(ko),
                                     start=(ko == 0), stop=(ko == KO - 1))
                    if ko == KO - 1:
                        evict1(hA[:, f, :], p1s[f][:], f % 2 == 0)

            # --- GEMM1 phase 2: fo-major, B's groups interleaved -------
            def gemm1B(f):
                pb = psB.tile([P, TB], f32, name="pb", tag="pB")
                for ko in range(KO):
                    nc.tensor.matmul(pb[:], w1_ap(f, ko), xB_ap(ko),
                                     start=(ko == 0), stop=(ko == KO - 1))
                nc.vector.tensor_scalar_max(hB[:, f, :], pb[:], 0.0)

            bq = list(range(FO)) if TB else []
            NB = len(bq)
            nA2 = max(FO - nfo1, 1)
            for i, f in enumerate(range(nfo1, FO)):
                p1 = psA.tile([P, TA], f32, name="p1", tag="pA")
                for ko in range(KO):
                    nc.tensor.matmul(p1[:], w1_ap(f, ko), xA_ap(ko),
                                     start=(ko == 0), stop=(ko == KO - 1))
                evict1(hA[:, f, :], p1[:], True)
                ntake = ((i + 1) * NB) // nA2 - (i * NB) // nA2
                for _ in range(ntake):
                    gemm1B(bq.pop(0))
            for f in bq:
                gemm1B(f)

            # --- GEMM2 --------------------------------------------------
            def gemm2A(do, c0, c1, ysb, use_act=True, dma_eng=None):
                p2 = psA.tile([P, TA], f32, name="p2", tag="pA")
                for f in range(FO):
                    nc.tensor.matmul(p2[:, 0:c1 - c0],
                                     w2sb[:, do, f * P:(f + 1) * P],
                                     hA[:, f, c0:c1],
                                     start=(f == 0), stop=(f == FO - 1))
                if use_act:
                    nc.scalar.copy(ysb[:], p2[:, 0:c1 - c0])
                else:
                    nc.vector.tensor_scalar_add(ysb[:], p2[:, 0:c1 - c0], 0.0)
                (dma_eng or nc.sync).dma_start(
                    yt.ap()[:, do * TA + c0:do * TA + c1], ysb[:])

            def gemm2B(do, ysbB):
                pb = psB.tile([P, TB], f32, name="p2b", tag="pB")
                for f in range(FO):
                    nc.tensor.matmul(pb[:],
                                     w2sb[:, do, f * P:(f + 1) * P],
                                     hB[:, f, :],
                                     start=(f == 0), stop=(f == FO - 1))
                nc.vector.tensor_scalar_add(
                    ysbB[:, do * TB:(do + 1) * TB], pb[:], 0.0)

            # B's tiny groups woven between the A d-groups (hides their
            # PE.SEQ decode behind long A matmuls); all B work and its
            # store finish during do=2, well before the scatter tail.
            ysbB = (ypool.tile([P, KO * TB], bf16, tag="yB", name="yB")
                    if TB else None)
            for do in range(KO - 1):
                ysb = ypool.tile([P, TA], bf16, tag="yA", name="yA")
                gemm2A(do, 0, TA, ysb)
                if TB:
                    gemm2B(do, ysbB)
                    if do == KO - 2:
                        gemm2B(KO - 1, ysbB)
                        nc.sync.dma_start(yt.ap()[:, KO * TA:KO * C],
                                          ysbB[:])

            if use_sw:
                # Last d-group via pre-generated SWDGE descriptors: the
                # trigger skips HWDGE desc-gen and the DGE->DMA handoff,
                # so the final store starts right after its eviction.
                p2 = psA.tile([P, TA], f32, name="p2s0", tag="pA")
                for f in range(FO):
                    nc.tensor.matmul(p2[:, 0:S0],
                                     w2sb[:, KO - 1, f * P:(f + 1) * P],
                                     hA[:, f, 0:S0],
                                     start=(f == 0), stop=(f == FO - 1))
                e0 = nc.scalar.copy(ysl0[:, 0, :], p2[:, 0:S0])
                # Tile encodes the trigger's deferred RAW dep on the
                # eviction as a standalone EventSemaphore wait parked on
                # the Pool sequencer right before the trigger — no extra
                # guard needed.
                nc.gpsimd.trigger_dma(count=None, queue_num=0)
                # Unpin the prep from the eviction's stream position so
                # its ~1us Pool desc-gen runs early, not between the
                # evict and the trigger.  Safe: desc-gen only reads idxs;
                # the data read happens at trigger time, which still
                # waits on the eviction.
                prep0.ins.try_remove_dependency(e0.ins.name)
                p2b = psA.tile([P, TA], f32, name="p2s1", tag="pA")
                for f in range(FO):
                    nc.tensor.matmul(p2b[:, 0:S1A],
                                     w2sb[:, KO - 1, f * P:(f + 1) * P],
                                     hA[:, f, S0:S0 + S1A],
                                     start=(f == 0), stop=(f == FO - 1))
                e1 = nc.vector.tensor_scalar_add(ysl1a[:, 0, :],
                                                 p2b[:, 0:S1A], 0.0)
                nc.gpsimd.trigger_dma(count=None, queue_num=1)
                prep1.ins.try_remove_dependency(e1.ins.name)
                p2c = psA.tile([P, TA], f32, name="p2s2", tag="pA")
                for f in range(FO):
                    nc.tensor.matmul(p2c[:, 0:S1B],
                                     w2sb[:, KO - 1, f * P:(f + 1) * P],
                                     hA[:, f, S0 + S1A:TA],
                                     start=(f == 0), stop=(f == FO - 1))
                e2 = nc.vector.tensor_scalar_add(ysl1b[:, 0, :],
                                                 p2c[:, 0:S1B], 0.0)
                nc.gpsimd.trigger_dma(count=None, queue_num=2)
                prep2.ins.try_remove_dependency(e2.ins.name)
                # No explicit wait on sem0/sem1: Tile's teardown drain
                # already waits for the prep DMA-completion sems (and the
                # scheduler would hoist a bare wait_ge above the triggers,
                # deadlocking the Pool queue).
            else:
                # fallback: column-split HWDGE stores
                subs = [TA - TA // 4, TA // 4] if TA >= 256 else [TA]
                c0 = 0
                for s, sub in enumerate(subs):
                    ysb = ypool.tile([P, sub], bf16, tag="yA3", name="yA3")
                    gemm2A(KO - 1, c0, c0 + sub, ysb,
                           use_act=(s % 2 == 0),
                           dma_eng=nc.scalar if s < len(subs) - 1 else None)
                    c0 += sub

    nc.compile()
    if use_sw:
        _mirror_inc_swdge_updates(nc)
        _strip_trigger_ticks(nc)
        _fold_trigger_waits(nc)
    _CACHE[key] = (nc, use_sw)
    return _CACHE[key]


def _fold_trigger_waits(nc):
    """Move each trigger's evict wait from its standalone EventSemaphore
    onto the trigger itself.

    Tile renders the trigger's deferred RAW dep as a separate Pool
    EventSemaphore directly before it, adding a sequencer hop (~100ns)
    to the tail critical path.  Fold the wait into the trigger (its ISA
    encoding carries exactly one wait, so this replaces the prep
    desc-gen gate — satisfied ~14us earlier: the preps generate on the
    idle Pool engine around 18us while the in-order sequencer cannot
    reach a trigger before its eviction at >31us).  The standalone
    becomes a waitless no-op that retires early.
    """
    import concourse.mybir as mybir

    fn = nc.m.functions[0]
    last_es = None
    for blk in fn.blocks:
        for ins in blk.instructions:
            nm = type(ins).__name__
            eng = getattr(ins, "engine", None)
            if eng is None or str(eng) != "EngineType.Pool":
                continue
            si = ins.sync_info
            if nm == "InstEventSemaphore":
                last_es = ins if si and len(si.on_wait) == 1 else None
            elif nm == "InstTriggerDma":
                if last_es is None:
                    continue
                esi = last_es.sync_info
                ups = list(si.on_update) if si else []
                ins.sync_info = mybir.SyncInfo(
                    on_wait=list(esi.on_wait), on_update=ups)
                last_es.sync_info = mybir.SyncInfo(
                    on_wait=[],
                    on_update=list(esi.on_update) if esi else [])
                last_es = None
            elif nm not in ("InstISA",):
                last_es = None


def _strip_trigger_ticks(nc):
    """Drop the trigger_dma sequencer-tick semaphores and their teardown
    wait.

    Each trigger's tick update is modeled as a DMA-completion semaphore
    (+~900ns propagation), and the teardown barrier waits for all of
    them — serializing the epilogue behind trigger_time+900 although the
    in-order Pool sequencer already guarantees the triggers issued
    before the teardown instructions that follow them.  With the wait
    gone the barrier overlaps the final store's completion window.
    Only strip when the accounting matches exactly (every wait on the
    tick semaphore expects precisely the total stripped ticks).
    """
    import concourse.mybir as mybir

    fn = nc.m.functions[0]
    tick_ids = {}
    for blk in fn.blocks:
        for ins in blk.instructions:
            si = ins.sync_info
            if not si or type(ins).__name__ != "InstTriggerDma":
                continue
            for u in si.on_update:
                tick_ids[u.id] = tick_ids.get(u.id, 0) + 1
    if not tick_ids:
        return
    # verify every wait on a tick sem expects the full count
    for blk in fn.blocks:
        for ins in blk.instructions:
            si = ins.sync_info
            if not si:
                continue
            for w in si.on_wait:
                if w.id in tick_ids and w.wait_value != tick_ids[w.id]:
                    return  # unexpected mid-stream wait: leave untouched
    for blk in fn.blocks:
        for ins in blk.instructions:
            si = ins.sync_info
            if not si:
                continue
            is_trig = type(ins).__name__ == "InstTriggerDma"
            ups = [u for u in si.on_update
                   if not (is_trig and u.id in tick_ids)]
            ws = [w for w in si.on_wait if w.id not in tick_ids]
            if len(ups) != len(si.on_update) or len(ws) != len(si.on_wait):
                ins.sync_info = mybir.SyncInfo(on_wait=ws, on_update=ups)


def _mirror_inc_swdge_updates(nc):
    """Expose InstIncSwdgeSem's payload-encoded semaphore bumps as
    sync_info updates.

    Tile's teardown reconciles the SWDGE DMA-lane semaphores with
    InstIncSwdgeSem bumps whose sems live in the instruction payload,
    not in sync_info.  The timeline cost model only sees sync_info, so
    without this mirror the final barrier waits on the lane sems and
    the simulation deadlocks.  The duplicate update is harmless for
    execution: the waits are >= and the teardown range-clears the sems.
    """
    import concourse.mybir as mybir
    from concourse import bass_isa

    for blk in nc.m.functions[0].blocks:
        for ins in blk.instructions:
            if not isinstance(ins, bass_isa.InstIncSwdgeSem):
                continue
            if ins._mode != "add":
                continue
            ups = list(ins.sync_info.on_update) if ins.sync_info else []
            for i, (val, nm) in enumerate(
                    zip(ins._sem_values, ins._sem_names)):
                if val:
                    ups.append(mybir.SyncUpdate(
                        sync_type="semaphore", id=ins._sem_id_base + i,
                        update_mode="sem-add-imm", update_value=val,
                        ant_name=nm))
            waits = list(ins.sync_info.on_wait) if ins.sync_info else []
            ins.sync_info = mybir.SyncInfo(on_wait=waits, on_update=ups)


_last = {}


def _pack_inputs(xs, w_htoh4, w_h4toh, idx_split, C, KO, FO, use_sw):
    bf16 = ml_dtypes.bfloat16
    chunks = _chunks_for(C)
    TA = chunks[0]
    TB = chunks[1] if len(chunks) > 1 else 0
    nfo1 = min(FO1, FO)
    RS = TA + nfo1 * P
    d_model = KO * P
    # idx i lives at [i % 16, i // 16]; the 16-partition wrap must be
    # replicated across all partition groups — the Q7 core serving SWDGE
    # queue k reads a channel stripe that depends on k.
    sidx_h = np.empty((P, 8), dtype=np.int16)
    for p in range(P):
        for s in range(8):
            sidx_h[p, s] = s * 16 + (p % 16)
    in_maps = []
    for e in range(NUM_EXPERT):
        idx = idx_split[e]
        cnt = len(idx)
        xT = np.zeros((d_model, C), dtype=np.float32)
        if cnt:
            xT[:, :cnt] = xs[idx].T
        xk = xT.reshape(KO, P, C)                          # [ko, p, c]
        w1t = w_htoh4[e].T.reshape(KO, P, FO, P)          # [ko, p, fo, f]
        rows = []
        for ko in range(KO):
            rows.append(xk[ko, :, :TA])                   # x-ko  (P, TA)
            rows.append(w1t[ko, :, :nfo1, :].reshape(P, nfo1 * P))
        xw_h = np.concatenate(rows, axis=1)               # (P, KO*RS)
        if TB:
            xB = xk[:, :, TA:C].transpose(1, 0, 2).reshape(P, KO * TB)
            xw_h = np.concatenate([xw_h, xB], axis=1)
        w1b_h = w1t[:, :, nfo1:, :].transpose(1, 2, 0, 3) \
            .reshape(P, FO - nfo1, KO * P)
        w2t = w_h4toh[e].T.reshape(FO, P, KO, P)          # [fo, p, do, d]
        w2_h = w2t.transpose(1, 2, 0, 3).reshape(P, KO, FO * P)
        m = {
            "xw": np.ascontiguousarray(xw_h.astype(bf16)),
            "w1b": np.ascontiguousarray(w1b_h.astype(bf16)),
            "w2": np.ascontiguousarray(w2_h.astype(bf16)),
        }
        if use_sw:
            m["sidx"] = sidx_h
        in_maps.append(m)
    return in_maps


def kernel(inp, gate_idx, gate_score, w_htoh4, w_h4toh):
    inp = np.ascontiguousarray(np.asarray(inp, dtype=np.float32))
    gate_idx = np.asarray(gate_idx)
    gate_score = np.asarray(gate_score, dtype=np.float32)
    w_htoh4 = np.asarray(w_htoh4, dtype=np.float32)
    w_h4toh = np.asarray(w_h4toh, dtype=np.float32)

    B, d_model = inp.shape
    n_expert, d_ff, _ = w_htoh4.shape
    assert n_expert == NUM_EXPERT
    KO = d_model // P
    FO = d_ff // P

    gi = gate_idx.astype(np.int64)
    order = np.argsort(gi, kind="stable")
    counts = np.bincount(gi, minlength=NUM_EXPERT)
    idx_split = np.split(order, np.cumsum(counts)[:-1])

    # exact capacity: every padded column costs a full matmul-group
    # column (KO*FO + FO*KO rows) on the bottleneck core
    C = max(int(counts.max()), 256)
    TA = _chunks_for(C)[0]

    scores_flat = gate_score.reshape(-1)
    xs = inp * scores_flat[:, None]

    nc, use_sw = _build(C, KO, FO)
    in_maps = _pack_inputs(xs, w_htoh4, w_h4toh, idx_split, C, KO, FO,
                           use_sw)

    from concourse import bass_utils
    res = bass_utils.run_bass_kernel_spmd(nc, in_maps,
                                          core_ids=list(range(N_CORES)))

    _last.update(nc=nc, in_maps=in_maps, res=res, C=C, KO=KO, FO=FO)

    y_full = np.empty((B, d_model), dtype=np.float32)
    for e in range(NUM_EXPERT):
        idx = idx_split[e]
        if len(idx) == 0:
            continue
        yt_h = res.results[e]["yt"].astype(np.float32)  # (P, KO*C)
        yA = yt_h[:, :KO * TA].reshape(P, KO, TA)
        if use_sw:
            yl = res.results[e]["ylast"].astype(np.float32)
            ylb = res.results[e]["ylastb"].astype(np.float32)
            _ss = os.environ.get("K_SS", "352,128,32")
            _s0, _s1a, _s1b = (int(v) for v in _ss.split(","))
            if _s0 + _s1a + _s1b != TA or min(_s0, _s1a) < 64 or _s1b < 32:
                _s0, _s1a, _s1b = TA - 128, 64, 64
            yl[:, _s0:_s0 + _s1a] = ylb[:, :_s1a]
            yl[:, TA - _s1b:TA] = ylb[:, 128:128 + _s1b]
            yA = np.concatenate(
                [yA[:, :KO - 1, :], yl[:, None, :]], axis=1)
        if C > TA:
            yB = yt_h[:, KO * TA:].reshape(P, KO, C - TA)
            yk = np.concatenate([yA, yB], axis=2)
        else:
            yk = yA
        yT = yk.transpose(1, 0, 2).reshape(d_model, C)
        y_full[idx] = yT[:, :len(idx)].T
    out = y_full[0::2] + y_full[1::2]
    return np.ascontiguousarray(out, dtype=np.float32)
